# revision 1
# baseline (speedup 1.0000x reference)
"""GRU encoder (Keras GRU v2, reset_after=True) on 8 Trainium2 NeuronCores.

Data-parallel on the batch axis: each core owns 16 of 128 batch rows and
runs the full computation for its shard:

  P1: embedding gather via dma_gather (bf16, transposed output) directly
      into [emb_k, token] layout, then one GEMM (bf16 inputs, fp32 accum)
      with the input bias folded in -> xp[token, 3H] in DRAM scratch,
      tokens stored t-major so step t's slice is contiguous.
  P2: 256 sequential GRU steps. rec = h @ U as fp32r matmuls (full-rate
      at N=512) with U resident in SBUF and h kept transposed (hT) via
      PE transposes each step. Gates on DVE/ACT:
        z = sigmoid(xz + rec_z); r = sigmoid(xr + rec_r)
        hh = tanh(xh + r * rec_h); h' = hh + z * (h - hh)

Inputs arrive full-size; sharding/layout prep happens on host; outputs
are gathered back to full shape. Everything numeric runs on device.
"""

from contextlib import ExitStack

import numpy as np
import ml_dtypes

B, S, VOCAB, EMB, H = 128, 256, 32000, 256, 1024
NC = 8
BL = B // NC
H3 = 3 * H
KC_E = EMB // 128
KC_H = H // 128


def _apply_tile_drain_patch():
    """This walrus build rejects >1 sync waits on a Drain (TPB_CTRL
    NO_STRUCT): spread the TileContext exit-drain waits over single-wait
    NoOps emitted just before the drain."""
    import bass_rust
    import concourse.tile as tile_mod
    from concourse.vector_clock import ScopedClock

    if getattr(tile_mod.TileContext, "_drain_patch_applied", False):
        return

    def _patched(self, tick_clock, wait_clock):
        nc = self.nc
        collector = nc.sync.nop(nofuse=True, hint="drain_wait_collector")
        wait_clock.add_sem_waits(
            collector.ins, ScopedClock({None: tick_clock.global_clock})
        )
        si = collector.ins.sync_info
        waits = list(si.on_wait) if si is not None else []
        if len(waits) > 1:
            si.on_wait = waits[:1]
            for i in range(1, len(waits)):
                extra = nc.sync.nop(nofuse=True, hint=f"drain_wait_{i}")
                extra.ins.sync_info = bass_rust.SyncInfo(
                    on_wait=waits[i : i + 1], on_update=[]
                )
        nc.sync.drain()
        nc.all_engine_barrier()
        assert self.sems is not None
        popped = nc._tile_sem_poison_stack.pop()
        assert popped is self._sem_poison
        nc.clear_and_free_semaphores(list(self.sems.allocated().values()))
        nc.all_engine_barrier()

    tile_mod.TileContext._drain_and_barrier = _patched
    tile_mod.TileContext._drain_patch_applied = True


def build(with_brec=False):
    _apply_tile_drain_patch()
    import concourse.mybir as mybir
    import concourse.tile as tile
    from concourse import bacc

    F32, F32R, BF16, I16 = (
        mybir.dt.float32,
        mybir.dt.float32r,
        mybir.dt.bfloat16,
        mybir.dt.int16,
    )
    AL = mybir.AluOpType
    AF = mybir.ActivationFunctionType

    steps = S
    nc = bacc.Bacc()
    TOK = BL * steps

    idx = nc.dram_tensor("idx", [BL, steps], I16, kind="ExternalInput")
    emb = nc.dram_tensor("emb", [VOCAB, EMB], BF16, kind="ExternalInput")
    wk = nc.dram_tensor("wk", [128, KC_E, H3], BF16, kind="ExternalInput")
    bias = nc.dram_tensor("bias", [1, H3], F32, kind="ExternalInput")
    uk = nc.dram_tensor("uk", [128, KC_H, H3], F32R, kind="ExternalInput")
    h0 = nc.dram_tensor("h0", [BL, H], F32, kind="ExternalInput")
    h0t = nc.dram_tensor("h0t", [128, KC_H, BL], F32R, kind="ExternalInput")
    ident = nc.dram_tensor("ident", [128, 128], F32, kind="ExternalInput")
    brec = None
    if with_brec:
        brec = nc.dram_tensor("brec", [1, H], F32, kind="ExternalInput")

    out = nc.dram_tensor("out", [BL, steps, H], F32, kind="ExternalOutput")
    state = nc.dram_tensor("state", [BL, H], F32, kind="ExternalOutput")
    xp_dram = nc.dram_tensor("xp_scratch", [TOK, H3], F32)

    with tile.TileContext(nc) as tc, ExitStack() as stack:
        persist = stack.enter_context(tc.tile_pool(name="persist", bufs=1))
        u_sb = persist.tile([128, KC_H, H3], F32R)
        nc.sync.dma_start(out=u_sb[:], in_=uk[:])
        id_sb = persist.tile([128, 128], F32)
        nc.sync.dma_start(out=id_sb[:], in_=ident[:])
        brec_sb = None
        if with_brec:
            brec_sb = persist.tile([128, H], F32)
            nc.gpsimd.dma_start(
                out=brec_sb[:], in_=brec[:].to_broadcast((128, H))
            )

        # ------------- P1: embedding gather + input projection -------------
        with (
            tc.tile_pool(name="p1", bufs=1) as p1,
            tc.tile_pool(name="p1sb", bufs=3) as p1sb,
            tc.tile_pool(name="p1ps", bufs=2, space="PSUM") as p1ps,
        ):
            bias_sb = p1.tile([128, H3], F32)
            nc.gpsimd.dma_start(
                out=bias_sb[:], in_=bias[:].to_broadcast((128, H3))
            )
            idx_sb = p1.tile([128, TOK // 16], I16)
            nc.vector.memset(idx_sb[:], 0)
            # token j = t*16 + b -> idx_sb[j % 16, j // 16] = idx[b, t];
            # the gather ucode runs on 8 Q7 cores, each reading its own
            # 16-partition replica of the index block.
            for qc in range(8):
                nc.sync.dma_start(
                    out=idx_sb[16 * qc : 16 * (qc + 1), :], in_=idx[:]
                )

            w_sb = p1.tile([128, KC_E, H3], BF16)
            nc.sync.dma_start(out=w_sb[:], in_=wk[:])

            G = min(512, TOK)  # >512 idxs per dma_gather crashes HW
            xeT_blocks = []
            for g in range(0, TOK, G):
                xeT_g = p1.tile([128, KC_E, G], BF16, tag=f"xe{g}")
                nc.gpsimd.dma_gather(
                    out_ap=xeT_g[:],
                    in_ap=emb[:],
                    idxs_ap=idx_sb[:, g // 16 : (g + G) // 16],
                    num_idxs=G,
                    num_idxs_reg=G,
                    elem_size=EMB,
                    transpose=True,
                )
                xeT_blocks.append(xeT_g)
            MB = G // 128
            for m in range(TOK // 128):
                xeT = xeT_blocks[m // MB]
                ml = m % MB
                for n in range(H3 // 512):
                    ps = p1ps.tile([128, 512], F32)
                    for kc in range(KC_E):
                        nc.tensor.matmul(
                            ps[:],
                            lhsT=xeT[:, kc, 128 * ml : 128 * (ml + 1)],
                            rhs=w_sb[:, kc, 512 * n : 512 * (n + 1)],
                            start=(kc == 0),
                            stop=(kc == KC_E - 1),
                        )
                    xp_sb = p1sb.tile([128, 512], F32)
                    nc.vector.tensor_tensor(
                        xp_sb[:], ps[:],
                        bias_sb[:, 512 * n : 512 * (n + 1)], AL.add,
                    )
                    nc.sync.dma_start(
                        out=xp_dram[
                            128 * m : 128 * (m + 1), 512 * n : 512 * (n + 1)
                        ],
                        in_=xp_sb[:],
                    )

        # ------------- P2: recurrence --------------------------------------
        with (
            tc.tile_pool(name="ht", bufs=2) as ht_pool,
            tc.tile_pool(name="hprev", bufs=2) as h_pool,
            tc.tile_pool(name="xpt", bufs=2) as xp_pool,
            tc.tile_pool(name="gates", bufs=2) as g_pool,
            tc.tile_pool(name="recps", bufs=1, space="PSUM") as rec_pool,
            tc.tile_pool(name="trps", bufs=2, space="PSUM") as tr_pool,
        ):
            hT = ht_pool.tile([128, KC_H, BL], F32R, tag="ht")
            nc.sync.dma_start(out=hT[:], in_=h0t[:])
            h_prev = h_pool.tile([BL, H], F32, tag="h")
            nc.sync.dma_start(out=h_prev[:], in_=h0[:])

            for t in range(steps):
                xp_t = xp_pool.tile([BL, H3], F32, tag="xp")
                nc.sync.dma_start(
                    out=xp_t[:], in_=xp_dram[BL * t : BL * (t + 1), :]
                )
                rec = rec_pool.tile([BL, H3], F32, tag="rec")

                def do_bank(n):
                    lo = 512 * n
                    for kc in range(KC_H):
                        nc.tensor.matmul(
                            rec[:, lo : lo + 512],
                            lhsT=hT[:, kc, :],
                            rhs=u_sb[:, kc, lo : lo + 512],
                            start=(kc == 0),
                            stop=(kc == KC_H - 1),
                        )

                # bank order: r gates, h gates, z gates — shortens the
                # per-step tail (m1/m2/hh chain overlaps the z matmuls)
                for n in (2, 3):
                    do_bank(n)
                r_t = g_pool.tile([BL, H], F32, tag="r")
                for j in (0, 1):
                    sl = slice(512 * j, 512 * (j + 1))
                    hi = slice(1024 + 512 * j, 1024 + 512 * (j + 1))
                    nc.vector.tensor_tensor(
                        r_t[:, sl], rec[:, hi], xp_t[:, hi], AL.add
                    )
                    nc.scalar.activation(r_t[:, sl], r_t[:, sl], AF.Sigmoid)
                for n in (4, 5):
                    do_bank(n)
                hh_t = g_pool.tile([BL, H], F32, tag="hh")
                m2_t = g_pool.tile([BL, H], F32, tag="m2")
                for j in (0, 1):
                    sl = slice(512 * j, 512 * (j + 1))
                    hi = slice(2048 + 512 * j, 2048 + 512 * (j + 1))
                    rh = rec[:, hi]
                    if with_brec:
                        tmp = g_pool.tile([BL, H], F32, tag="tmpb")
                        nc.vector.tensor_tensor(
                            tmp[:, sl], rh, brec_sb[:BL, sl], AL.add
                        )
                        rh = tmp[:, sl]
                    nc.vector.tensor_tensor(m2_t[:, sl], r_t[:, sl], rh, AL.mult)
                    nc.vector.tensor_tensor(
                        m2_t[:, sl], m2_t[:, sl], xp_t[:, hi], AL.add
                    )
                    nc.scalar.activation(hh_t[:, sl], m2_t[:, sl], AF.Tanh)
                for n in (0, 1):
                    do_bank(n)
                z_t = g_pool.tile([BL, H], F32, tag="z")
                for j in (0, 1):
                    sl = slice(512 * j, 512 * (j + 1))
                    nc.vector.tensor_tensor(
                        z_t[:, sl], rec[:, sl], xp_t[:, sl], AL.add
                    )
                    nc.scalar.activation(z_t[:, sl], z_t[:, sl], AF.Sigmoid)

                h_new = h_pool.tile([BL, H], F32, tag="h")
                d_t = g_pool.tile([BL, H], F32, tag="d")
                for j in (0, 1):
                    sl = slice(512 * j, 512 * (j + 1))
                    nc.vector.tensor_tensor(
                        d_t[:, sl], h_prev[:, sl], hh_t[:, sl], AL.subtract
                    )
                    nc.vector.tensor_tensor(
                        d_t[:, sl], z_t[:, sl], d_t[:, sl], AL.mult
                    )
                    nc.vector.tensor_tensor(
                        h_new[:, sl], hh_t[:, sl], d_t[:, sl], AL.add
                    )

                nc.sync.dma_start(out=out[:, t, :], in_=h_new[:])
                if t == steps - 1:
                    nc.sync.dma_start(out=state[:], in_=h_new[:])
                else:
                    hT_next = ht_pool.tile([128, KC_H, BL], F32R, tag="ht")
                    for kc in range(KC_H):
                        trp = tr_pool.tile([128, BL], F32, tag="tr")
                        nc.tensor.transpose(
                            trp[:],
                            h_new[:, 128 * kc : 128 * (kc + 1)],
                            id_sb[:BL, :BL],
                        )
                        nc.vector.tensor_copy(hT_next[:, kc, :], trp[:])
                    hT = hT_next
                h_prev = h_new
    nc.finalize()
    return nc


def host_inputs(x, hidden, emb, W, U, b):
    x = np.asarray(x).astype(np.int64)
    hidden = np.asarray(hidden, np.float32)
    emb = np.asarray(emb, np.float32)
    W = np.asarray(W, np.float32)
    U = np.asarray(U, np.float32)
    b = np.asarray(b, np.float32)

    emb_bf = emb.astype(ml_dtypes.bfloat16)
    wk = np.ascontiguousarray(
        W.reshape(KC_E, 128, H3).transpose(1, 0, 2)
    ).astype(ml_dtypes.bfloat16)
    bias_v = (
        b[0] + np.concatenate([b[1][: 2 * H], np.zeros(H, np.float32)])
    )[None, :].astype(np.float32)
    uk = np.ascontiguousarray(
        U.reshape(KC_H, 128, H3).transpose(1, 0, 2)
    ).astype(np.float32)
    identity = np.eye(128, dtype=np.float32)
    with_brec = bool(np.any(b[1][2 * H :]))
    brec_v = b[1][2 * H :][None, :].astype(np.float32)

    in_maps = []
    for c in range(NC):
        rows = slice(BL * c, BL * (c + 1))
        h0 = np.ascontiguousarray(hidden[rows])
        h0t = np.ascontiguousarray(h0.T.reshape(KC_H, 128, BL).transpose(1, 0, 2))
        m = {
            "idx": np.ascontiguousarray(x[rows]).astype(np.int16),
            "emb": emb_bf,
            "wk": wk,
            "bias": bias_v,
            "uk": uk,
            "h0": h0,
            "h0t": h0t,
            "ident": identity,
        }
        if with_brec:
            m["brec"] = brec_v
        in_maps.append(m)
    return in_maps, with_brec


_cache = {}


def kernel(x, hidden, emb, W, U, b):
    from concourse.bass_utils import run_bass_kernel_spmd

    in_maps, with_brec = host_inputs(x, hidden, emb, W, U, b)
    if with_brec not in _cache:
        _cache[with_brec] = build(with_brec=with_brec)
    nc = _cache[with_brec]
    res = run_bass_kernel_spmd(nc, in_maps, core_ids=list(range(NC)))
    output = np.concatenate([r["out"] for r in res.results], axis=0)
    output = np.ascontiguousarray(output).reshape(B, S, H)
    state = np.concatenate([r["state"] for r in res.results], axis=0)
    return output, state


# revision 2
# speedup vs baseline: 1.0281x; 1.0281x over previous
"""GRU encoder (Keras GRU v2, reset_after=True) on 8 Trainium2 NeuronCores.

Data-parallel on the batch axis: each core owns 16 of 128 batch rows and
runs the full computation for its shard:

  P1: embedding gather via dma_gather (bf16, transposed output) directly
      into [emb_k, token] layout, then one GEMM (bf16 inputs, fp32 accum)
      with the input bias folded in -> xp[token, 3H] in DRAM scratch,
      tokens stored t-major so step t's slice is contiguous.
  P2: 256 sequential GRU steps. rec = h @ U as fp32r matmuls (full-rate
      at N=512) with U resident in SBUF and h kept transposed (hT) via
      PE transposes each step. Gates on DVE/ACT:
        z = sigmoid(xz + rec_z); r = sigmoid(xr + rec_r)
        hh = tanh(xh + r * rec_h); h' = hh + z * (h - hh)

Inputs arrive full-size; sharding/layout prep happens on host; outputs
are gathered back to full shape. Everything numeric runs on device.
"""

from contextlib import ExitStack

import numpy as np
import ml_dtypes

B, S, VOCAB, EMB, H = 128, 256, 32000, 256, 1024
NC = 8
BL = B // NC
H3 = 3 * H
KC_E = EMB // 128
KC_H = H // 128


def _apply_tile_drain_patch():
    """This walrus build rejects >1 sync waits on a Drain (TPB_CTRL
    NO_STRUCT): spread the TileContext exit-drain waits over single-wait
    NoOps emitted just before the drain."""
    import bass_rust
    import concourse.tile as tile_mod
    from concourse.vector_clock import ScopedClock

    if getattr(tile_mod.TileContext, "_drain_patch_applied", False):
        return

    def _patched(self, tick_clock, wait_clock):
        nc = self.nc
        collector = nc.sync.nop(nofuse=True, hint="drain_wait_collector")
        wait_clock.add_sem_waits(
            collector.ins, ScopedClock({None: tick_clock.global_clock})
        )
        si = collector.ins.sync_info
        waits = list(si.on_wait) if si is not None else []
        if len(waits) > 1:
            si.on_wait = waits[:1]
            for i in range(1, len(waits)):
                extra = nc.sync.nop(nofuse=True, hint=f"drain_wait_{i}")
                extra.ins.sync_info = bass_rust.SyncInfo(
                    on_wait=waits[i : i + 1], on_update=[]
                )
        nc.sync.drain()
        nc.all_engine_barrier()
        assert self.sems is not None
        popped = nc._tile_sem_poison_stack.pop()
        assert popped is self._sem_poison
        nc.clear_and_free_semaphores(list(self.sems.allocated().values()))
        nc.all_engine_barrier()

    tile_mod.TileContext._drain_and_barrier = _patched
    tile_mod.TileContext._drain_patch_applied = True


def build(with_brec=False):
    _apply_tile_drain_patch()
    import concourse.mybir as mybir
    import concourse.tile as tile
    from concourse import bacc

    F32, F32R, BF16, I16 = (
        mybir.dt.float32,
        mybir.dt.float32r,
        mybir.dt.bfloat16,
        mybir.dt.int16,
    )
    AL = mybir.AluOpType
    AF = mybir.ActivationFunctionType

    steps = S
    nc = bacc.Bacc()
    TOK = BL * steps

    idx = nc.dram_tensor("idx", [BL, steps], I16, kind="ExternalInput")
    emb = nc.dram_tensor("emb", [VOCAB, EMB], BF16, kind="ExternalInput")
    wk = nc.dram_tensor("wk", [128, KC_E, H3], BF16, kind="ExternalInput")
    bias = nc.dram_tensor("bias", [1, H3], F32, kind="ExternalInput")
    uk = nc.dram_tensor("uk", [128, KC_H, H3], F32R, kind="ExternalInput")
    h0 = nc.dram_tensor("h0", [BL, H], F32, kind="ExternalInput")
    h0t = nc.dram_tensor("h0t", [128, KC_H, BL], F32R, kind="ExternalInput")
    ident = nc.dram_tensor("ident", [128, 128], F32, kind="ExternalInput")
    brec = None
    if with_brec:
        brec = nc.dram_tensor("brec", [1, H], F32, kind="ExternalInput")

    out = nc.dram_tensor("out", [BL, steps, H], F32, kind="ExternalOutput")
    state = nc.dram_tensor("state", [BL, H], F32, kind="ExternalOutput")
    xp_dram = nc.dram_tensor("xp_scratch", [TOK, H3], F32R)

    with tile.TileContext(nc) as tc, ExitStack() as stack:
        persist = stack.enter_context(tc.tile_pool(name="persist", bufs=1))
        u_sb = persist.tile([128, KC_H, H3], F32R)
        nc.sync.dma_start(out=u_sb[:], in_=uk[:])
        id_sb = persist.tile([128, 128], F32)
        nc.sync.dma_start(out=id_sb[:], in_=ident[:])
        idr_sb = persist.tile([BL, BL], F32R)
        nc.vector.tensor_copy(idr_sb[:], id_sb[:BL, :BL])
        brec_sb = None
        if with_brec:
            brec_sb = persist.tile([128, H], F32)
            nc.gpsimd.dma_start(
                out=brec_sb[:], in_=brec[:].to_broadcast((128, H))
            )

        # ------------- P1: embedding gather + input projection -------------
        with (
            tc.tile_pool(name="p1", bufs=1) as p1,
            tc.tile_pool(name="p1sb", bufs=3) as p1sb,
            tc.tile_pool(name="p1ps", bufs=2, space="PSUM") as p1ps,
        ):
            bias_sb = p1.tile([128, H3], F32)
            nc.gpsimd.dma_start(
                out=bias_sb[:], in_=bias[:].to_broadcast((128, H3))
            )
            idx_sb = p1.tile([128, TOK // 16], I16)
            nc.vector.memset(idx_sb[:], 0)
            # token j = t*16 + b -> idx_sb[j % 16, j // 16] = idx[b, t];
            # the gather ucode runs on 8 Q7 cores, each reading its own
            # 16-partition replica of the index block.
            for qc in range(8):
                nc.sync.dma_start(
                    out=idx_sb[16 * qc : 16 * (qc + 1), :], in_=idx[:]
                )

            w_sb = p1.tile([128, KC_E, H3], BF16)
            nc.sync.dma_start(out=w_sb[:], in_=wk[:])

            G = min(512, TOK)  # >512 idxs per dma_gather crashes HW
            xeT_blocks = []
            for g in range(0, TOK, G):
                xeT_g = p1.tile([128, KC_E, G], BF16, tag=f"xe{g}")
                nc.gpsimd.dma_gather(
                    out_ap=xeT_g[:],
                    in_ap=emb[:],
                    idxs_ap=idx_sb[:, g // 16 : (g + G) // 16],
                    num_idxs=G,
                    num_idxs_reg=G,
                    elem_size=EMB,
                    transpose=True,
                )
                xeT_blocks.append(xeT_g)
            MB = G // 128
            for m in range(TOK // 128):
                xeT = xeT_blocks[m // MB]
                ml = m % MB
                for n in range(H3 // 512):
                    ps = p1ps.tile([128, 512], F32)
                    for kc in range(KC_E):
                        nc.tensor.matmul(
                            ps[:],
                            lhsT=xeT[:, kc, 128 * ml : 128 * (ml + 1)],
                            rhs=w_sb[:, kc, 512 * n : 512 * (n + 1)],
                            start=(kc == 0),
                            stop=(kc == KC_E - 1),
                        )
                    xp_sb = p1sb.tile([128, 512], F32R)
                    nc.vector.tensor_tensor(
                        xp_sb[:], ps[:],
                        bias_sb[:, 512 * n : 512 * (n + 1)], AL.add,
                    )
                    nc.sync.dma_start(
                        out=xp_dram[
                            128 * m : 128 * (m + 1), 512 * n : 512 * (n + 1)
                        ],
                        in_=xp_sb[:],
                    )

        # ------------- P2: recurrence --------------------------------------
        with (
            tc.tile_pool(name="ht", bufs=2) as ht_pool,
            tc.tile_pool(name="hprev", bufs=2) as h_pool,
            tc.tile_pool(name="xpt", bufs=3) as xp_pool,
            tc.tile_pool(name="gates", bufs=2) as g_pool,
            tc.tile_pool(name="recps", bufs=1, space="PSUM") as rec_pool,
            tc.tile_pool(name="trps", bufs=2, space="PSUM") as tr_pool,
        ):
            hT = ht_pool.tile([128, KC_H, BL], F32R, tag="ht")
            nc.sync.dma_start(out=hT[:], in_=h0t[:])
            h_prev = h_pool.tile([BL, H], F32, tag="h")
            nc.sync.dma_start(out=h_prev[:], in_=h0[:])

            for t in range(steps):
                xp_t = xp_pool.tile([BL, H3], F32R, tag="xp")
                nc.sync.dma_start(
                    out=xp_t[:], in_=xp_dram[BL * t : BL * (t + 1), :]
                )
                rec = rec_pool.tile([BL, H3], F32, tag="rec")

                def do_bank(n, add_xp=False):
                    lo = 512 * n
                    if add_xp:
                        nc.tensor.matmul(
                            rec[:, lo : lo + 512],
                            lhsT=idr_sb[:],
                            rhs=xp_t[:, lo : lo + 512],
                            start=True,
                            stop=False,
                        )
                    for kc in range(KC_H):
                        nc.tensor.matmul(
                            rec[:, lo : lo + 512],
                            lhsT=hT[:, kc, :],
                            rhs=u_sb[:, kc, lo : lo + 512],
                            start=(not add_xp and kc == 0),
                            stop=(kc == KC_H - 1),
                        )

                # bank order: r gates, h gates, z gates — shortens the
                # per-step tail (m1/m2/hh chain overlaps the z matmuls)
                for n in (2, 3):
                    do_bank(n, add_xp=True)
                r_t = g_pool.tile([BL, H], F32, tag="r")
                for j in (0, 1):
                    sl = slice(512 * j, 512 * (j + 1))
                    hi = slice(1024 + 512 * j, 1024 + 512 * (j + 1))
                    nc.scalar.activation(r_t[:, sl], rec[:, hi], AF.Sigmoid)
                for n in (4, 5):
                    do_bank(n)
                hh_t = g_pool.tile([BL, H], F32, tag="hh")
                m2_t = g_pool.tile([BL, H], F32, tag="m2")
                for j in (0, 1):
                    sl = slice(512 * j, 512 * (j + 1))
                    hi = slice(2048 + 512 * j, 2048 + 512 * (j + 1))
                    rh = rec[:, hi]
                    if with_brec:
                        tmp = g_pool.tile([BL, H], F32, tag="tmpb")
                        nc.vector.tensor_tensor(
                            tmp[:, sl], rh, brec_sb[:BL, sl], AL.add
                        )
                        rh = tmp[:, sl]
                    nc.vector.tensor_tensor(m2_t[:, sl], r_t[:, sl], rh, AL.mult)
                    nc.vector.tensor_tensor(
                        m2_t[:, sl], m2_t[:, sl],
                        xp_t[:, hi].bitcast(F32), AL.add,
                    )
                    nc.scalar.activation(hh_t[:, sl], m2_t[:, sl], AF.Tanh)
                for n in (0, 1):
                    do_bank(n, add_xp=True)
                z_t = g_pool.tile([BL, H], F32, tag="z")
                for j in (0, 1):
                    sl = slice(512 * j, 512 * (j + 1))
                    nc.scalar.activation(z_t[:, sl], rec[:, sl], AF.Sigmoid)

                h_new = h_pool.tile([BL, H], F32, tag="h")
                d_t = g_pool.tile([BL, H], F32, tag="d")
                for j in (0, 1):
                    sl = slice(512 * j, 512 * (j + 1))
                    nc.vector.tensor_tensor(
                        d_t[:, sl], h_prev[:, sl], hh_t[:, sl], AL.subtract
                    )
                    nc.vector.tensor_tensor(
                        d_t[:, sl], z_t[:, sl], d_t[:, sl], AL.mult
                    )
                    nc.vector.tensor_tensor(
                        h_new[:, sl], hh_t[:, sl], d_t[:, sl], AL.add
                    )

                nc.sync.dma_start(out=out[:, t, :], in_=h_new[:])
                if t == steps - 1:
                    nc.sync.dma_start(out=state[:], in_=h_new[:])
                else:
                    hT_next = ht_pool.tile([128, KC_H, BL], F32R, tag="ht")
                    for kc in range(KC_H):
                        trp = tr_pool.tile([128, BL], F32, tag="tr")
                        nc.tensor.transpose(
                            trp[:],
                            h_new[:, 128 * kc : 128 * (kc + 1)],
                            id_sb[:BL, :BL],
                        )
                        nc.vector.tensor_copy(hT_next[:, kc, :], trp[:])
                    hT = hT_next
                h_prev = h_new
    nc.finalize()
    return nc


def host_inputs(x, hidden, emb, W, U, b):
    x = np.asarray(x).astype(np.int64)
    hidden = np.asarray(hidden, np.float32)
    emb = np.asarray(emb, np.float32)
    W = np.asarray(W, np.float32)
    U = np.asarray(U, np.float32)
    b = np.asarray(b, np.float32)

    emb_bf = emb.astype(ml_dtypes.bfloat16)
    wk = np.ascontiguousarray(
        W.reshape(KC_E, 128, H3).transpose(1, 0, 2)
    ).astype(ml_dtypes.bfloat16)
    bias_v = (
        b[0] + np.concatenate([b[1][: 2 * H], np.zeros(H, np.float32)])
    )[None, :].astype(np.float32)
    uk = np.ascontiguousarray(
        U.reshape(KC_H, 128, H3).transpose(1, 0, 2)
    ).astype(np.float32)
    identity = np.eye(128, dtype=np.float32)
    with_brec = bool(np.any(b[1][2 * H :]))
    brec_v = b[1][2 * H :][None, :].astype(np.float32)

    in_maps = []
    for c in range(NC):
        rows = slice(BL * c, BL * (c + 1))
        h0 = np.ascontiguousarray(hidden[rows])
        h0t = np.ascontiguousarray(h0.T.reshape(KC_H, 128, BL).transpose(1, 0, 2))
        m = {
            "idx": np.ascontiguousarray(x[rows]).astype(np.int16),
            "emb": emb_bf,
            "wk": wk,
            "bias": bias_v,
            "uk": uk,
            "h0": h0,
            "h0t": h0t,
            "ident": identity,
        }
        if with_brec:
            m["brec"] = brec_v
        in_maps.append(m)
    return in_maps, with_brec


_cache = {}


def kernel(x, hidden, emb, W, U, b):
    from concourse.bass_utils import run_bass_kernel_spmd

    in_maps, with_brec = host_inputs(x, hidden, emb, W, U, b)
    if with_brec not in _cache:
        _cache[with_brec] = build(with_brec=with_brec)
    nc = _cache[with_brec]
    res = run_bass_kernel_spmd(nc, in_maps, core_ids=list(range(NC)))
    output = np.concatenate([r["out"] for r in res.results], axis=0)
    output = np.ascontiguousarray(output).reshape(B, S, H)
    state = np.concatenate([r["state"] for r in res.results], axis=0)
    return output, state


# revision 3
# speedup vs baseline: 1.4383x; 1.3990x over previous
"""GRU encoder (Keras GRU v2, reset_after=True) on 8 Trainium2 NeuronCores.

Data-parallel on the batch axis: each core owns 16 of 128 batch rows and
runs the full computation for its shard:

  P1: embedding gather via dma_gather (bf16, transposed output) directly
      into [emb_k, token] layout, then one GEMM (bf16 inputs, fp32 accum)
      with the input bias folded in -> xp[token, 3H] in DRAM scratch,
      tokens stored t-major so step t's slice is contiguous.
  P2: 256 sequential GRU steps. rec = h @ U as fp32r matmuls (full-rate
      at N=512) with U resident in SBUF and h kept transposed (hT) via
      PE transposes each step. Gates on DVE/ACT:
        z = sigmoid(xz + rec_z); r = sigmoid(xr + rec_r)
        hh = tanh(xh + r * rec_h); h' = hh + z * (h - hh)

Inputs arrive full-size; sharding/layout prep happens on host; outputs
are gathered back to full shape. Everything numeric runs on device.
"""

from contextlib import ExitStack

import numpy as np
import ml_dtypes

B, S, VOCAB, EMB, H = 128, 256, 32000, 256, 1024
NC = 8
BL = B // NC
H3 = 3 * H
KC_E = EMB // 128
KC_H = H // 128


def _apply_tile_drain_patch():
    """This walrus build rejects >1 sync waits on a Drain (TPB_CTRL
    NO_STRUCT): spread the TileContext exit-drain waits over single-wait
    NoOps emitted just before the drain."""
    import bass_rust
    import concourse.tile as tile_mod
    from concourse.vector_clock import ScopedClock

    if getattr(tile_mod.TileContext, "_drain_patch_applied", False):
        return

    def _patched(self, tick_clock, wait_clock):
        nc = self.nc
        collector = nc.sync.nop(nofuse=True, hint="drain_wait_collector")
        wait_clock.add_sem_waits(
            collector.ins, ScopedClock({None: tick_clock.global_clock})
        )
        si = collector.ins.sync_info
        waits = list(si.on_wait) if si is not None else []
        if len(waits) > 1:
            si.on_wait = waits[:1]
            for i in range(1, len(waits)):
                extra = nc.sync.nop(nofuse=True, hint=f"drain_wait_{i}")
                extra.ins.sync_info = bass_rust.SyncInfo(
                    on_wait=waits[i : i + 1], on_update=[]
                )
        nc.sync.drain()
        nc.all_engine_barrier()
        assert self.sems is not None
        popped = nc._tile_sem_poison_stack.pop()
        assert popped is self._sem_poison
        nc.clear_and_free_semaphores(list(self.sems.allocated().values()))
        nc.all_engine_barrier()

    tile_mod.TileContext._drain_and_barrier = _patched
    tile_mod.TileContext._drain_patch_applied = True


def build(with_brec=False):
    _apply_tile_drain_patch()
    import concourse.mybir as mybir
    import concourse.tile as tile
    from concourse import bacc

    F32, F32R, BF16, I16 = (
        mybir.dt.float32,
        mybir.dt.float32r,
        mybir.dt.bfloat16,
        mybir.dt.int16,
    )
    AL = mybir.AluOpType
    AF = mybir.ActivationFunctionType

    steps = S
    nc = bacc.Bacc()
    TOK = BL * steps

    idx = nc.dram_tensor("idx", [BL, steps], I16, kind="ExternalInput")
    emb = nc.dram_tensor("emb", [VOCAB, EMB], BF16, kind="ExternalInput")
    wk = nc.dram_tensor("wk", [128, KC_E, H3], BF16, kind="ExternalInput")
    bias = nc.dram_tensor("bias", [1, H3], F32, kind="ExternalInput")
    uk = nc.dram_tensor("uk", [128, KC_H, H3], F32R, kind="ExternalInput")
    h0 = nc.dram_tensor("h0", [BL, H], F32, kind="ExternalInput")
    h0t = nc.dram_tensor("h0t", [128, KC_H, BL], F32R, kind="ExternalInput")
    ident = nc.dram_tensor("ident", [128, 128], F32, kind="ExternalInput")
    brec = None
    if with_brec:
        brec = nc.dram_tensor("brec", [1, H], F32, kind="ExternalInput")

    out = nc.dram_tensor("out", [BL, steps, H], F32, kind="ExternalOutput")
    state = nc.dram_tensor("state", [BL, H], F32, kind="ExternalOutput")
    xp_dram = nc.dram_tensor("xp_scratch", [TOK, H3], F32R)

    with tile.TileContext(nc) as tc, ExitStack() as stack:
        persist = stack.enter_context(tc.tile_pool(name="persist", bufs=1))
        u_sb = persist.tile([128, KC_H, H3], F32R)
        nc.sync.dma_start(out=u_sb[:], in_=uk[:])
        id_sb = persist.tile([128, 128], F32)
        nc.sync.dma_start(out=id_sb[:], in_=ident[:])
        idr_sb = persist.tile([BL, BL], F32R)
        nc.vector.tensor_copy(idr_sb[:], id_sb[:BL, :BL])
        brec_sb = None
        if with_brec:
            brec_sb = persist.tile([128, H], F32)
            nc.gpsimd.dma_start(
                out=brec_sb[:], in_=brec[:].to_broadcast((128, H))
            )

        # ------------- P1: embedding gather + input projection -------------
        with (
            tc.tile_pool(name="p1", bufs=1) as p1,
            tc.tile_pool(name="p1sb", bufs=3) as p1sb,
            tc.tile_pool(name="p1ps", bufs=2, space="PSUM") as p1ps,
        ):
            bias_sb = p1.tile([128, H3], F32)
            nc.gpsimd.dma_start(
                out=bias_sb[:], in_=bias[:].to_broadcast((128, H3))
            )
            idx_sb = p1.tile([128, TOK // 16], I16)
            nc.vector.memset(idx_sb[:], 0)
            # token j = t*16 + b -> idx_sb[j % 16, j // 16] = idx[b, t];
            # the gather ucode runs on 8 Q7 cores, each reading its own
            # 16-partition replica of the index block.
            for qc in range(8):
                nc.sync.dma_start(
                    out=idx_sb[16 * qc : 16 * (qc + 1), :], in_=idx[:]
                )

            w_sb = p1.tile([128, KC_E, H3], BF16)
            nc.sync.dma_start(out=w_sb[:], in_=wk[:])

            G = min(512, TOK)  # >512 idxs per dma_gather crashes HW
            xeT_blocks = []
            for g in range(0, TOK, G):
                xeT_g = p1.tile([128, KC_E, G], BF16, tag=f"xe{g}")
                nc.gpsimd.dma_gather(
                    out_ap=xeT_g[:],
                    in_ap=emb[:],
                    idxs_ap=idx_sb[:, g // 16 : (g + G) // 16],
                    num_idxs=G,
                    num_idxs_reg=G,
                    elem_size=EMB,
                    transpose=True,
                )
                xeT_blocks.append(xeT_g)
            MB = G // 128
            for m in range(TOK // 128):
                xeT = xeT_blocks[m // MB]
                ml = m % MB
                for n in range(H3 // 512):
                    ps = p1ps.tile([128, 512], F32)
                    for kc in range(KC_E):
                        nc.tensor.matmul(
                            ps[:],
                            lhsT=xeT[:, kc, 128 * ml : 128 * (ml + 1)],
                            rhs=w_sb[:, kc, 512 * n : 512 * (n + 1)],
                            start=(kc == 0),
                            stop=(kc == KC_E - 1),
                        )
                    xp_sb = p1sb.tile([128, 512], F32R)
                    nc.vector.tensor_tensor(
                        xp_sb[:], ps[:],
                        bias_sb[:, 512 * n : 512 * (n + 1)], AL.add,
                    )
                    nc.sync.dma_start(
                        out=xp_dram[
                            128 * m : 128 * (m + 1), 512 * n : 512 * (n + 1)
                        ],
                        in_=xp_sb[:],
                    )

        # ------------- P2: recurrence --------------------------------------
        with (
            tc.tile_pool(name="ht", bufs=2) as ht_pool,
            tc.tile_pool(name="hprev", bufs=2) as h_pool,
            tc.tile_pool(name="xpt", bufs=3) as xp_pool,
            tc.tile_pool(name="gates", bufs=2) as g_pool,
            tc.tile_pool(name="recps", bufs=1, space="PSUM") as rec_pool,
            tc.tile_pool(name="trps", bufs=2, space="PSUM") as tr_pool,
        ):
            hT = ht_pool.tile([128, KC_H, BL], F32R, tag="ht")
            nc.sync.dma_start(out=hT[:], in_=h0t[:])
            h_prev = h_pool.tile([BL, H], F32, tag="h")
            nc.sync.dma_start(out=h_prev[:], in_=h0[:])

            for t in range(steps):
                xp_t = xp_pool.tile([BL, H3], F32R, tag="xp")
                nc.sync.dma_start(
                    out=xp_t[:], in_=xp_dram[BL * t : BL * (t + 1), :]
                )
                rec = rec_pool.tile([BL, H3], F32, tag="rec")

                def do_bank(n, add_xp=False):
                    lo = 512 * n
                    if add_xp:
                        nc.tensor.matmul(
                            rec[:, lo : lo + 512],
                            lhsT=idr_sb[:],
                            rhs=xp_t[:, lo : lo + 512],
                            start=True,
                            stop=False,
                        )
                    for kc in range(KC_H):
                        nc.tensor.matmul(
                            rec[:, lo : lo + 512],
                            lhsT=hT[:, kc, :],
                            rhs=u_sb[:, kc, lo : lo + 512],
                            start=(not add_xp and kc == 0),
                            stop=(kc == KC_H - 1),
                        )

                # bank order: r gates, h gates, z gates — shortens the
                # per-step tail (m1/m2/hh chain overlaps the z matmuls)
                for n in (2, 3):
                    do_bank(n, add_xp=True)
                r_t = g_pool.tile([BL, H], F32, tag="r")
                for j in (0, 1):
                    sl = slice(512 * j, 512 * (j + 1))
                    hi = slice(1024 + 512 * j, 1024 + 512 * (j + 1))
                    nc.scalar.activation(r_t[:, sl], rec[:, hi], AF.Sigmoid)
                for n in (4, 5):
                    do_bank(n)
                hh_t = g_pool.tile([BL, H], F32, tag="hh")
                m2_t = g_pool.tile([BL, H], F32, tag="m2")
                d_t = g_pool.tile([BL, H], F32, tag="d")
                h_new = h_pool.tile([BL, H], F32, tag="h")
                for j in (0, 1):
                    sl = slice(512 * j, 512 * (j + 1))
                    hi = slice(2048 + 512 * j, 2048 + 512 * (j + 1))
                    rh = rec[:, hi]
                    if with_brec:
                        tmp = g_pool.tile([BL, H], F32, tag="tmpb")
                        nc.vector.tensor_tensor(
                            tmp[:, sl], rh, brec_sb[:BL, sl], AL.add
                        )
                        rh = tmp[:, sl]
                    nc.vector.tensor_tensor(m2_t[:, sl], r_t[:, sl], rh, AL.mult)
                    nc.vector.tensor_tensor(
                        m2_t[:, sl], m2_t[:, sl],
                        xp_t[:, hi].bitcast(F32), AL.add,
                    )
                    nc.scalar.activation(hh_t[:, sl], m2_t[:, sl], AF.Tanh)
                    nc.vector.tensor_tensor(
                        d_t[:, sl], h_prev[:, sl], hh_t[:, sl], AL.subtract
                    )
                for n in (0, 1):
                    do_bank(n, add_xp=True)
                z_t = g_pool.tile([BL, H], F32, tag="z")
                hT_next = (
                    None
                    if t == steps - 1
                    else ht_pool.tile([128, KC_H, BL], F32R, tag="ht")
                )
                for j in (0, 1):
                    sl = slice(512 * j, 512 * (j + 1))
                    nc.scalar.activation(z_t[:, sl], rec[:, sl], AF.Sigmoid)
                    nc.vector.tensor_tensor(
                        d_t[:, sl], z_t[:, sl], d_t[:, sl], AL.mult
                    )
                    nc.vector.tensor_tensor(
                        h_new[:, sl], hh_t[:, sl], d_t[:, sl], AL.add
                    )
                    if hT_next is not None:
                        for kc in range(4 * j, 4 * (j + 1)):
                            trp = tr_pool.tile([128, BL], F32, tag="tr")
                            nc.tensor.transpose(
                                trp[:],
                                h_new[:, 128 * kc : 128 * (kc + 1)],
                                id_sb[:BL, :BL],
                            )
                            nc.vector.tensor_copy(hT_next[:, kc, :], trp[:])
                nc.sync.dma_start(out=out[:, t, :], in_=h_new[:])
                if t == steps - 1:
                    nc.sync.dma_start(out=state[:], in_=h_new[:])
                else:
                    hT = hT_next
                h_prev = h_new
    nc.finalize()
    return nc


def host_inputs(x, hidden, emb, W, U, b):
    x = np.asarray(x).astype(np.int64)
    hidden = np.asarray(hidden, np.float32)
    emb = np.asarray(emb, np.float32)
    W = np.asarray(W, np.float32)
    U = np.asarray(U, np.float32)
    b = np.asarray(b, np.float32)

    emb_bf = emb.astype(ml_dtypes.bfloat16)
    wk = np.ascontiguousarray(
        W.reshape(KC_E, 128, H3).transpose(1, 0, 2)
    ).astype(ml_dtypes.bfloat16)
    bias_v = (
        b[0] + np.concatenate([b[1][: 2 * H], np.zeros(H, np.float32)])
    )[None, :].astype(np.float32)
    uk = np.ascontiguousarray(
        U.reshape(KC_H, 128, H3).transpose(1, 0, 2)
    ).astype(np.float32)
    identity = np.eye(128, dtype=np.float32)
    with_brec = bool(np.any(b[1][2 * H :]))
    brec_v = b[1][2 * H :][None, :].astype(np.float32)

    in_maps = []
    for c in range(NC):
        rows = slice(BL * c, BL * (c + 1))
        h0 = np.ascontiguousarray(hidden[rows])
        h0t = np.ascontiguousarray(h0.T.reshape(KC_H, 128, BL).transpose(1, 0, 2))
        m = {
            "idx": np.ascontiguousarray(x[rows]).astype(np.int16),
            "emb": emb_bf,
            "wk": wk,
            "bias": bias_v,
            "uk": uk,
            "h0": h0,
            "h0t": h0t,
            "ident": identity,
        }
        if with_brec:
            m["brec"] = brec_v
        in_maps.append(m)
    return in_maps, with_brec


_cache = {}


def kernel(x, hidden, emb, W, U, b):
    from concourse.bass_utils import run_bass_kernel_spmd

    in_maps, with_brec = host_inputs(x, hidden, emb, W, U, b)
    if with_brec not in _cache:
        _cache[with_brec] = build(with_brec=with_brec)
    nc = _cache[with_brec]
    res = run_bass_kernel_spmd(nc, in_maps, core_ids=list(range(NC)))
    output = np.concatenate([r["out"] for r in res.results], axis=0)
    output = np.ascontiguousarray(output).reshape(B, S, H)
    state = np.concatenate([r["state"] for r in res.results], axis=0)
    return output, state
